# revision 1
# baseline (speedup 1.0000x reference)
"""2D Haar DWT (level 1) Trainium2 Bass kernel.

Input  x: [16, 64, 256, 256] f32
Output y: [16, 256, 128, 128] f32, y[n, s*64+c, i, j] = Haar mix s of the
2x2 block x[n, c, 2i:2i+2, 2j:2j+2].

Sharding: pure data parallel over the batch dim — core k gets batches
[2k, 2k+2).

Per-core design (memory-bound, ~67 MB HBM traffic/core, ~190 us roofline):

Oct-row layout: a group of G=4 channel planes (1 MB, contiguous in DRAM) is
loaded so SBUF partition p = (c*32 + row//8) holds 8 consecutive rows — a
pure [128, 2048] reshape of the DRAM stream (8 KB contiguous runs, 2-dim
AP). Both Haar butterfly stages are then same-partition, unit/2-strided
VectorE ops:
  stage 1 (vertical):  sum/diff of row pairs  -> one sd tile [128,2,4,256]
  scale: ScalarE in-place *0.5 on sd (folds the Haar normalization)
  stage 2 (horizontal): sd even +/- odd cols  -> oadd [p,v,rh4,j] holds
       subbands 0 (v=0) and 1 (v=1); osub holds subbands 2 and 3.
Stores are [128, 512] -> 256 KB per subband pair slice with 2 KB contiguous
DRAM runs (2-dim AP, full 128 partitions): output row i = 4*(p%32) + rh4.

Engine budget/core: DVE 4 ops/group ~150 us, ACT ~66 us, DMA ~195-215 us
(bottleneck), PE unused (fp32 matmul runs at 1/4 rate — measured slower
than DVE for this transform).
"""

import sys

sys.path.insert(0, "/opt/trn_rl_repo")

import numpy as np

import concourse.bacc as bacc
import concourse.mybir as mybir
from concourse.tile import TileContext

N_CORES = 8
N_PER_CORE = 2  # batches per core
C = 64  # input channels
H = 256
W = 256
G = 8  # channels per group (2 MB loads, 16 rows/partition)
F32 = mybir.dt.float32


def build_nc():
    nc = bacc.Bacc("TRN2", target_bir_lowering=False, debug=False)
    x = nc.dram_tensor("x", [N_PER_CORE, C, H, W], F32, kind="ExternalInput")
    y = nc.dram_tensor("y", [N_PER_CORE, 4 * C, H // 2, W // 2], F32, kind="ExternalOutput")

    with TileContext(nc) as tc:
        with (
            tc.tile_pool(name="inpool", bufs=4) as inpool,
            tc.tile_pool(name="sdpool", bufs=3) as sdpool,
            tc.tile_pool(name="outpool", bufs=5) as outpool,
        ):
            gi = 0
            for n in range(N_PER_CORE):
                for c0 in range(0, C, G):
                    # --- load: pure reshape of the 1 MB contiguous group.
                    # it[p, o, w] = x[n, c0 + p//32, 8*(p%32) + o, w]
                    it = inpool.tile([128, G * 512], F32, tag="in")
                    src = x[n, c0 : c0 + G].rearrange("c (q o) w -> (c q) o w", o=2 * G)
                    nc.sync.dma_start(
                        out=it[:].rearrange("p (o w) -> p o w", o=2 * G), in_=src
                    )

                    # --- stage 1 (vertical): rows 2t / 2t+1 within a partition
                    itv = it[:].rearrange("p (r t w) -> p r t w", r=G, t=2)
                    sd = sdpool.tile([128, G * 512], F32, tag="sd")
                    sdv = sd[:].rearrange("p (v r w) -> p v r w", v=2, r=G)
                    nc.vector.tensor_add(
                        out=sdv[:, 0], in0=itv[:, :, 0, :], in1=itv[:, :, 1, :]
                    )
                    nc.vector.tensor_sub(
                        out=sdv[:, 1], in0=itv[:, :, 0, :], in1=itv[:, :, 1, :]
                    )

                    # --- Haar 0.5 normalization: ScalarE scales only the odd
                    # columns in place; the even-column 0.5 folds into stage 2
                    # via scalar_tensor_tensor ((in0*0.5) +/- in1).
                    sdj = sd[:].rearrange("p (v r j t) -> p v r j t", v=2, r=G, t=2)
                    nc.scalar.mul(sdj[..., 1], sdj[..., 1], 0.5)

                    # --- stage 2 (horizontal): even/odd column butterfly
                    oadd = outpool.tile([128, G * 256], F32, tag="oadd")
                    osub = outpool.tile([128, G * 256], F32, tag="osub")
                    oadd_v = oadd[:].rearrange("p (v r j) -> p v r j", v=2, r=G)
                    osub_v = osub[:].rearrange("p (v r j) -> p v r j", v=2, r=G)
                    nc.vector.scalar_tensor_tensor(
                        out=oadd_v,
                        in0=sdj[..., 0],
                        scalar=0.5,
                        in1=sdj[..., 1],
                        op0=mybir.AluOpType.mult,
                        op1=mybir.AluOpType.add,
                    )
                    nc.vector.scalar_tensor_tensor(
                        out=osub_v,
                        in0=sdj[..., 0],
                        scalar=0.5,
                        in1=sdj[..., 1],
                        op0=mybir.AluOpType.mult,
                        op1=mybir.AluOpType.subtract,
                    )

                    # --- stores: (tile, v, subband); i = 4*(p%32) + rh4, so the
                    # DRAM side is [p step 512][(r j) 512] — 2 KB runs.
                    for t_, v, s in ((oadd, 0, 0), (oadd, 1, 1), (osub, 0, 2), (osub, 1, 3)):
                        dst = y[n, s * C + c0 : s * C + c0 + G].rearrange(
                            "c (q r) j -> (c q) (r j)", r=G
                        )
                        eng = nc.sync if (gi * 4 + s) % 2 == 0 else nc.scalar
                        eng.dma_start(
                            out=dst,
                            in_=t_[:].rearrange("p (v f) -> p v f", v=2)[:, v],
                        )
                    gi += 1

    nc.finalize()
    return nc


_NC = None


def _get_nc():
    global _NC
    if _NC is None:
        _NC = build_nc()
    return _NC


def kernel(x: np.ndarray) -> np.ndarray:
    from concourse.bass_utils import run_bass_kernel_spmd

    x = np.ascontiguousarray(np.asarray(x), dtype=np.float32)
    assert x.shape == (16, C, H, W), x.shape

    nc = _get_nc()
    in_maps = [
        {"x": x[k * N_PER_CORE : (k + 1) * N_PER_CORE]} for k in range(N_CORES)
    ]
    res = run_bass_kernel_spmd(nc, in_maps, core_ids=list(range(N_CORES)))
    return np.concatenate([r["y"] for r in res.results], axis=0)

